# revision 1
# baseline (speedup 1.0000x reference)
"""Trainium2 Bass kernel for sliding-window (window=256) causal attention.

Model (B=1, S=4096, H=1024, nh=16, hd=64, no q-scaling):
  q,k,v = x@wq.T, x@wk.T, x@wv.T ; scores = q@k.T (banded causal window 256)
  out = softmax(scores)@v reassembled, then @wo.T + bo

Sharding: 2 heads per core across 8 cores (tensor parallel on the head dim).
Each core computes a partial output  ctx_c @ wo[:, c-slice].T  (+ bias on
core 0 only); the host sums the 8 partials (the all-reduce step).

Per-core pipeline (everything in "transposed" layouts to keep the PE fed):
  P1: xT tiles via PE transpose-mode; QT/KT/VT = w.T.T @ xT  (fp32r matmuls)
  P1.5: V tiles via PE transpose of VT; stored bf16 as [V_h0 | 0 | V_h1]
  P2: per key-tile kt: scoresT = K@Q.T for q-window of 3 tiles (fp32r),
      band mask added on the PE (identity-matmul accumulate of a mask tile),
      exp on ACT (psum -> bf16 sbuf);
      ctx/denominator accumulated per q-tile via zero-padded stationaries so
      both heads + both quantities land partition-aligned in one psum bank;
      normalize with one reciprocal + one multiply; out-projection (fp32r)
      + bias add; DMA out.
"""

import numpy as np

import concourse.bass as bass
import concourse.tile as tile
from concourse import bacc, mybir
from concourse.bass_utils import run_bass_kernel_spmd

S = 4096
H = 1024
NH = 16
HD = 64
WIN = 256
N_CORES = 8
HEADS_PER_CORE = NH // N_CORES  # 2
CD = HEADS_PER_CORE * HD  # 128 ctx dims per core
NEG = -1e30

F32 = mybir.dt.float32
F32R = mybir.dt.float32r
BF16 = mybir.dt.bfloat16

N_ST = S // 128  # 32 s-tiles
N_KT = H // 128  # 8 contraction tiles for projections
N_SC = S // 512  # 8 s-chunks for projections


def _r(ap):
    return ap.bitcast(F32R)


def build_program(taps=False, reps=1):
    nc = bacc.Bacc("TRN2", target_bir_lowering=False, debug=False)
    tap_aps = {}
    if taps:
        for nm, shp in (("qt_d", [128, S]), ("kt_d", [128, S]), ("vt_d", [128, S]),
                        ("ex_d", [128, 384]), ("sps_d", [128, 384]),
                        ("ctx_d", [128, 128]), ("stg_d", [128, 256])):
            tap_aps[nm] = nc.dram_tensor(nm, shp, F32, kind="ExternalOutput").ap()

    x_ap = nc.dram_tensor("x", [S, H], F32R, kind="ExternalInput").ap()
    wqT_ap = nc.dram_tensor("wqT", [H, CD], F32R, kind="ExternalInput").ap()
    wkT_ap = nc.dram_tensor("wkT", [H, CD], F32R, kind="ExternalInput").ap()
    wvT_ap = nc.dram_tensor("wvT", [H, CD], F32R, kind="ExternalInput").ap()
    woT_ap = nc.dram_tensor("woT", [CD, H], F32R, kind="ExternalInput").ap()
    bo_ap = nc.dram_tensor("bo_b", [128, H], F32, kind="ExternalInput").ap()
    m3_ap = nc.dram_tensor("m3", [128, 384], F32R, kind="ExternalInput").ap()
    id_ap = nc.dram_tensor("ident", [128, 128], F32R, kind="ExternalInput").ap()
    out_ap = nc.dram_tensor("out", [S, H], F32, kind="ExternalOutput").ap()

    with tile.TileContext(nc) as tc:
        with (
            tc.tile_pool(name="consts", bufs=1) as consts,
            tc.tile_pool(name="big", bufs=1) as big,
        ):
            # ---- constant loads ----
            wq_sb = consts.tile([128, N_KT, CD], F32R)
            wk_sb = consts.tile([128, N_KT, CD], F32R)
            wv_sb = consts.tile([128, N_KT, CD], F32R)
            for w_sb, w_ap in ((wq_sb, wqT_ap), (wk_sb, wkT_ap), (wv_sb, wvT_ap)):
                nc.sync.dma_start(
                    out=w_sb[:], in_=w_ap.rearrange("(kt p) d -> p kt d", p=128)
                )
            wo_sb = consts.tile([128, H], F32R)
            nc.sync.dma_start(out=wo_sb[:], in_=woT_ap[:])
            bo_sb = consts.tile([128, H], F32)
            nc.sync.dma_start(out=bo_sb[:], in_=bo_ap[:])
            m3_sb = consts.tile([128, 384], F32R)
            nc.sync.dma_start(out=m3_sb[:], in_=m3_ap[:])
            id_sb = consts.tile([128, 128], F32R)
            nc.sync.dma_start(out=id_sb[:], in_=id_ap[:])
            # dn stationaries: [ones | zeros | ones]; h0 -> [0:128], h1 -> [64:192]
            on2 = consts.tile([128, 192], BF16)
            nc.gpsimd.memset(on2[:, 0:64], 1.0)
            nc.gpsimd.memset(on2[:, 64:128], 0.0)
            nc.gpsimd.memset(on2[:, 128:192], 1.0)

            # ---- persistent activations ----
            qt_sb = big.tile([128, S], F32R)  # QT: [2h*64 dims, S]
            kt_sb = big.tile([128, S], F32R)
            vt_sb = big.tile([128, S], F32R)
            # VA: per key-tile [V_h0(64) | zeros(64) | V_h1(64)] in bf16
            va = big.tile([128, N_ST, 192], BF16)
            nc.gpsimd.memset(va[:, :, 64:128], 0.0)
            # normalized ctx for all q-tiles (consumed by phase-3 out-proj)
            ctx_all = big.tile([128, N_ST, 128], F32R)

            for _rep in range(reps):
                # ================= Phase 1: xT + projections =================
                with (
                    tc.tile_pool(name="xstage", bufs=3) as xstage,
                    tc.tile_pool(name="xtc", bufs=2) as xtc,
                    tc.tile_pool(name="ps128", bufs=6, space="PSUM") as ps128,
                    tc.tile_pool(name="ps512", bufs=2, space="PSUM") as ps512,
                    tc.tile_pool(name="expp", bufs=4) as expp,
                    tc.tile_pool(name="stgp", bufs=2) as stgp,
                    tc.tile_pool(name="recp", bufs=2) as recp,
                    tc.tile_pool(name="outp", bufs=3) as outp,
                ):
                    cp_i = 0
                    for sc in range(N_SC):
                        xT_c = xtc.tile([128, N_KT, 512], F32R)
                        for st4 in range(4):
                            xst = xstage.tile([128, H], F32R)
                            row0 = (sc * 4 + st4) * 128
                            nc.sync.dma_start(out=xst[:], in_=x_ap[row0 : row0 + 128, :])
                            for kt in range(N_KT):
                                tp = ps128.tile([128, 128], F32R, tag="t128")
                                nc.tensor.transpose(
                                    tp[:], xst[:, kt * 128 : (kt + 1) * 128], id_sb[:]
                                )
                                dst = xT_c[:, kt, st4 * 128 : (st4 + 1) * 128]
                                if cp_i % 2 == 0:
                                    nc.vector.tensor_copy(dst, tp[:])
                                else:
                                    nc.scalar.copy(dst, tp[:])
                                cp_i += 1
                        for w_sb, dstT in ((wq_sb, qt_sb), (wk_sb, kt_sb), (wv_sb, vt_sb)):
                            pps = ps512.tile([128, 512], F32, tag="t512")
                            for kt in range(N_KT):
                                nc.tensor.matmul(
                                    pps[:],
                                    w_sb[:, kt, :],
                                    xT_c[:, kt, :],
                                    start=(kt == 0),
                                    stop=(kt == N_KT - 1),
                                )
                            dst = dstT[:, sc * 512 : (sc + 1) * 512]
                            if cp_i % 2 == 0:
                                nc.vector.tensor_copy(dst, pps[:])
                            else:
                                nc.scalar.copy(dst, pps[:])
                            cp_i += 1
                    if taps:
                        for nm, t in (("qt_d", qt_sb), ("kt_d", kt_sb), ("vt_d", vt_sb)):
                            stg_t = xstage.tile([128, H], F32)
                            for scc in range(4):
                                nc.vector.tensor_copy(stg_t[:], t[:, scc*1024:(scc+1)*1024].bitcast(F32))
                                nc.sync.dma_start(out=tap_aps[nm][:, scc*1024:(scc+1)*1024], in_=stg_t[:])
                    # ---- Phase 1.5: V tiles (transpose VT), bf16 ----
                    for kt in range(N_ST):
                        tp = ps128.tile([128, 128], F32R, tag="t128")
                        nc.tensor.transpose(
                            tp[:], vt_sb[:, kt * 128 : (kt + 1) * 128], id_sb[:]
                        )
                        nc.vector.tensor_copy(va[:, kt, 0:64], tp[:, 0:64])
                        nc.scalar.copy(va[:, kt, 128:192], tp[:, 64:128])

                    # ================= Phase 2: attention + out-proj =================
                    ctx_ps = {}
                    dn_ps = {}
                    for kt in range(N_ST):
                        W = min(384, (N_ST - kt) * 128)
                        q0 = kt * 128
                        # scoresT for both heads, then PE mask-add, then exp
                        sps_l = []
                        for h in (0, 1):
                            sps = ps512.tile([128, 384], F32, tag="t512")
                            nc.tensor.matmul(
                                sps[:, :W],
                                kt_sb[h * 64 : (h + 1) * 64, q0 : q0 + 128],
                                qt_sb[h * 64 : (h + 1) * 64, q0 : q0 + W],
                                start=True,
                                stop=False,
                            )
                            sps_l.append(sps)
                        for h in (0, 1):
                            nc.tensor.matmul(
                                sps_l[h][:, :W],
                                id_sb[:],
                                m3_sb[:, :W],
                                start=False,
                                stop=True,
                            )
                        ex_l = []
                        for h in (0, 1):
                            ex = expp.tile([128, 384], BF16)
                            nc.scalar.activation(
                                ex[:, :W],
                                sps_l[h][:, :W],
                                mybir.ActivationFunctionType.Exp,
                            )
                            ex_l.append(ex)
                        if taps and kt == 5:
                            tpt = stgp.tile([128, 384], F32)
                            nc.vector.tensor_copy(tpt[:], sps_l[0][:])
                            nc.sync.dma_start(out=tap_aps["sps_d"][:], in_=tpt[:])
                            tpt2 = stgp.tile([128, 384], F32)
                            nc.vector.tensor_copy(tpt2[:], ex_l[0][:])
                            nc.sync.dma_start(out=tap_aps["ex_d"][:], in_=tpt2[:])
                        # ctx + denominator accumulation per q-tile
                        for h in (0, 1):
                            va_h = va[:, kt, 0:128] if h == 0 else va[:, kt, 64:192]
                            on_h = on2[:, 0:128] if h == 0 else on2[:, 64:192]
                            for j in range(W // 128):
                                qt = kt + j
                                if qt not in ctx_ps:
                                    ctile = ps128.tile([128, 128], F32, tag="t128")
                                    ctx_ps[qt] = ctile
                                    dtile = ps128.tile([128, 128], F32, tag="t128")
                                    dn_ps[qt] = dtile
                                first = kt == max(qt - 2, 0) and h == 0
                                last = kt == qt and h == 1
                                rhs = ex_l[h][:, j * 128 : (j + 1) * 128]
                                nc.tensor.matmul(
                                    ctx_ps[qt][:], va_h, rhs, start=first, stop=last
                                )
                                nc.tensor.matmul(
                                    dn_ps[qt][:], on_h, rhs, start=first, stop=last
                                )
                        # finalize q-tile kt: normalize into ctx_all
                        qt = kt
                        stg = stgp.tile([128, 256], F32)
                        nc.scalar.copy(stg[:, 0:128], ctx_ps.pop(qt)[:])
                        nc.scalar.copy(stg[:, 128:256], dn_ps.pop(qt)[:])
                        rec = recp.tile([128, 128], F32)
                        nc.vector.reciprocal(rec[:], stg[:, 128:256])
                        nc.vector.tensor_mul(ctx_all[:, qt, :], stg[:, 0:128], rec[:])
                        if taps and qt == 5:
                            nc.sync.dma_start(out=tap_aps["ctx_d"][:], in_=ctx_all[:, qt, :].bitcast(F32))
                            nc.sync.dma_start(out=tap_aps["stg_d"][:], in_=stg[:])
                        osb = outp.tile([128, H], F32)
                        for half in range(2):
                            ops = ps512.tile([128, 512], F32, tag="t512")
                            nc.tensor.matmul(
                                ops[:],
                                ctx_all[:, qt, :],
                                wo_sb[:, half * 512 : (half + 1) * 512],
                                start=True,
                                stop=True,
                            )
                            nc.vector.tensor_add(
                                osb[:, half * 512 : (half + 1) * 512],
                                ops[:],
                                bo_sb[:, half * 512 : (half + 1) * 512],
                            )
                        nc.sync.dma_start(
                            out=out_ap[qt * 128 : (qt + 1) * 128, :], in_=osb[:]
                        )

    nc.compile()
    return nc


def build_in_maps(x, wq, wk, wv, wo, bo):
    xf = np.ascontiguousarray(x.reshape(S, H), dtype=np.float32)

    # band mask blocks in [k-part, q-free] tile coords, additive
    b = np.arange(128)[:, None]
    a = np.arange(128)[None, :]
    mask_a = np.where(b <= a, 0.0, NEG).astype(np.float32)  # diag tile (qt==kt)
    mask_b = np.where(b > a, 0.0, NEG).astype(np.float32)  # qt==kt+2 tile
    m3 = np.concatenate(
        [mask_a, np.zeros((128, 128), np.float32), mask_b], axis=1
    )
    ident = np.eye(128, dtype=np.float32)

    in_maps = []
    for c in range(N_CORES):
        r0, r1 = c * CD, (c + 1) * CD
        bo_b = np.broadcast_to(
            (bo if c == 0 else np.zeros_like(bo)).astype(np.float32), (128, H)
        ).copy()
        in_maps.append(
            {
                "x": xf,
                "wqT": np.ascontiguousarray(wq[r0:r1, :].T, dtype=np.float32),
                "wkT": np.ascontiguousarray(wk[r0:r1, :].T, dtype=np.float32),
                "wvT": np.ascontiguousarray(wv[r0:r1, :].T, dtype=np.float32),
                "woT": np.ascontiguousarray(wo[:, r0:r1].T, dtype=np.float32),
                "bo_b": bo_b,
                "m3": m3,
                "ident": ident,
            }
        )
    return in_maps


_NC_CACHE = None


def kernel(x, wq, wk, wv, wo, bo):
    global _NC_CACHE
    if _NC_CACHE is None:
        _NC_CACHE = build_program()
    nc = _NC_CACHE
    in_maps = build_in_maps(x, wq, wk, wv, wo, bo)
    res = run_bass_kernel_spmd(nc, in_maps, list(range(N_CORES)))
    out = res.results[0]["out"].astype(np.float64)
    for c in range(1, N_CORES):
        out += res.results[c]["out"]
    return out.reshape(1, S, H).astype(np.float32)



# revision 9
# speedup vs baseline: 1.2175x; 1.2175x over previous
"""Trainium2 Bass kernel for sliding-window (window=256) causal attention.

Model (B=1, S=4096, H=1024, nh=16, hd=64, no q-scaling):
  q,k,v = x@wq.T, x@wk.T, x@wv.T ; scores = q@k.T (banded causal window 256)
  out = softmax(scores)@v reassembled, then @wo.T + bo

Sharding: 2 heads per core across 8 cores (tensor parallel on the head dim).
Each core computes a partial output  ctx_c @ wo[:, c-slice].T  (+ bias on
core 0 only); the host sums the 8 bf16 partials (the all-reduce step).

v2 (bf16 operands, x pre-transposed on host, phases interleaved per chunk):
  chunk sc: QT/KT = w.T.T @ xT (fp32 psum, FD=512) -> bf16 sbuf;
            V direct in [s,d] orientation (accum over kt) -> bf16 sbuf.
  key-tile kt (interleaved, 4 per chunk):
      scoresT[k,q] h0/h1 as row-tiled concurrent matmuls into a 2-bank psum
      tile; band-mask added via identity-matmul on the two masked 128-wide
      slices only; one exp over both heads (ACT);
      ctx/dn accumulated per q-tile as col-tiled concurrent matmul pairs
      (h0 -> psum[0:64], h1 -> psum[64:128]); ctx+dn of two consecutive
      q-tiles share one psum bank (single accumulation group per bank);
      normalize with reciprocal+mul (DVE, direct from psum);
      out-projection (bf16 stationary, FD=512 x2, both halves in one
      2-bank psum tile) + bias: half0 fused into the DVE psum->sbuf add,
      half1 via ACT copy + GPSIMD bias add; DMA out bf16.

PSUM budget (8 banks): tsc pool [128,2,512] bufs=3 (6 banks, scores +
out-proj share) + tcd pool [128,512] bufs=2 (2 banks, phase-1 q/k/v psums
+ ctx/dn pairs share).
"""

import numpy as np
import ml_dtypes

import concourse.bass as bass
import concourse.tile as tile
from concourse import bacc, mybir
from concourse.bass_utils import run_bass_kernel_spmd

S = 4096
H = 1024
NH = 16
HD = 64
WIN = 256
N_CORES = 8
HEADS_PER_CORE = NH // N_CORES  # 2
CD = HEADS_PER_CORE * HD  # 128 ctx dims per core
NEG = -1e30

F32 = mybir.dt.float32
BF16 = mybir.dt.bfloat16

N_ST = S // 128  # 32 s-tiles
N_KT = H // 128  # 8 contraction tiles for projections
N_SC = S // 512  # 8 s-chunks for q/k projections
BF = ml_dtypes.bfloat16


def build_program(taps=False, reps=1):
    nc = bacc.Bacc("TRN2", target_bir_lowering=False, debug=False)

    xT_ap = nc.dram_tensor("xT_b", [128, N_KT, S], BF16, kind="ExternalInput").ap()
    wq_ap = nc.dram_tensor("wq_b", [128, N_KT, CD], BF16, kind="ExternalInput").ap()
    wk_ap = nc.dram_tensor("wk_b", [128, N_KT, CD], BF16, kind="ExternalInput").ap()
    wv_ap = nc.dram_tensor("wv_b", [128, N_KT, CD], BF16, kind="ExternalInput").ap()
    wo_ap = nc.dram_tensor("wo_b", [CD, H], BF16, kind="ExternalInput").ap()
    bo_ap = nc.dram_tensor("bo_b", [128, H], BF16, kind="ExternalInput").ap()
    m2_ap = nc.dram_tensor("m2_b", [128, 256], BF16, kind="ExternalInput").ap()
    id_ap = nc.dram_tensor("id_b", [128, 128], BF16, kind="ExternalInput").ap()
    out_ap = nc.dram_tensor("out", [S, H], BF16, kind="ExternalOutput").ap()

    with tile.TileContext(nc) as tc:
        with (
            tc.tile_pool(name="consts", bufs=1) as consts,
            tc.tile_pool(name="big", bufs=1) as big,
        ):
            # ---- constant loads ----
            wq_sb = consts.tile([128, N_KT, CD], BF16)
            wk_sb = consts.tile([128, N_KT, CD], BF16)
            wv_sb = consts.tile([128, N_KT, CD], BF16)
            for w_sb, w_ap in ((wq_sb, wq_ap), (wk_sb, wk_ap), (wv_sb, wv_ap)):
                nc.sync.dma_start(out=w_sb[:], in_=w_ap[:])
            wo_sb = consts.tile([128, H], BF16)
            nc.sync.dma_start(out=wo_sb[:], in_=wo_ap[:])
            bo_sb = consts.tile([128, H], BF16)
            nc.sync.dma_start(out=bo_sb[:], in_=bo_ap[:])
            m2_sb = consts.tile([128, 256], BF16)
            nc.sync.dma_start(out=m2_sb[:], in_=m2_ap[:])
            id_sb = consts.tile([128, 128], BF16)
            nc.sync.dma_start(out=id_sb[:], in_=id_ap[:])
            # dn stationaries: [ones | zeros | ones]; h0 -> 0:128, h1 -> 64:192
            on2 = consts.tile([128, 192], BF16)
            nc.gpsimd.memset(on2[:, 0:64], 1.0)
            nc.gpsimd.memset(on2[:, 64:128], 0.0)
            nc.gpsimd.memset(on2[:, 128:192], 1.0)

            # ---- persistent activations ----
            xT_sb = big.tile([128, N_KT, S], BF16)
            for sc in range(N_SC):
                s0 = sc * 512
                nc.sync.dma_start(
                    out=xT_sb[:, :, s0 : s0 + 512], in_=xT_ap[:, :, s0 : s0 + 512]
                )
            qt_sb = big.tile([128, S], BF16)  # QT: [2h*64 dims, S]
            kt_sb = big.tile([128, S], BF16)
            # V per s-tile, padded: [V_h0(64) | zeros(64) | V_h1(64)]
            va = big.tile([128, N_ST, 192], BF16)
            nc.gpsimd.memset(va[:, :, 64:128], 0.0)

            for _rep in range(reps):
                with (
                    tc.tile_pool(name="pssc", bufs=3, space="PSUM") as pssc,
                    tc.tile_pool(name="pscd", bufs=2, space="PSUM") as pscd,
                    tc.tile_pool(name="expp", bufs=3) as expp,
                    tc.tile_pool(name="recp", bufs=2) as recp,
                    tc.tile_pool(name="ctxp", bufs=3) as ctxp,
                    tc.tile_pool(name="outp", bufs=3) as outp,
                ):
                    cd_tiles = {}  # pair index -> psum tile [128, 512]
                    cp_state = [0]

                    def emit_chunk(sc):
                        s0 = sc * 512
                        for w_sb, dstT in ((wq_sb, qt_sb), (wk_sb, kt_sb)):
                            pps = pscd.tile([128, 512], F32, tag="tcd")
                            for kt in range(N_KT):
                                nc.tensor.matmul(
                                    pps[:],
                                    w_sb[:, kt, :],
                                    xT_sb[:, kt, s0 : s0 + 512],
                                    start=(kt == 0),
                                    stop=(kt == N_KT - 1),
                                )
                            dst = dstT[:, s0 : s0 + 512]
                            if cp_state[0] % 2 == 0:
                                nc.vector.tensor_copy(dst, pps[:])
                            else:
                                nc.scalar.copy(dst, pps[:])
                            cp_state[0] += 1
                        # V: direct [s, d] orientation per 128-row s-tile
                        for st4 in range(4):
                            st = sc * 4 + st4
                            r0 = st * 128
                            vps = pscd.tile([128, 512], F32, tag="tcd")
                            for kt in range(N_KT):
                                nc.tensor.matmul(
                                    vps[:, 0:128],
                                    xT_sb[:, kt, r0 : r0 + 128],
                                    wv_sb[:, kt, :],
                                    start=(kt == 0),
                                    stop=(kt == N_KT - 1),
                                )
                            nc.vector.tensor_copy(va[:, st, 0:64], vps[:, 0:64])
                            nc.scalar.copy(va[:, st, 128:192], vps[:, 64:128])
                            cp_state[0] += 1

                    def emit_kt(kt):
                        nj = min(3, N_ST - kt)
                        W = nj * 128
                        q0 = kt * 128
                        # scoresT both heads: row-tiled concurrent pair
                        sps = pssc.tile([128, 2, 512], F32, tag="tsc")
                        for h in (0, 1):
                            nc.tensor.matmul(
                                sps[:, h, 0:W],
                                kt_sb[h * 64 : (h + 1) * 64, q0 : q0 + 128],
                                qt_sb[h * 64 : (h + 1) * 64, q0 : q0 + W],
                                start=True,
                                stop=False,
                            )
                        # band mask: diag slice always, j2 slice when nj==3
                        for h in (0, 1):
                            nc.tensor.matmul(
                                sps[:, h, 0:128],
                                id_sb[:],
                                m2_sb[:, 0:128],
                                start=False,
                                stop=(nj < 3),
                            )
                            if nj == 3:
                                nc.tensor.matmul(
                                    sps[:, h, 256:384],
                                    id_sb[:],
                                    m2_sb[:, 128:256],
                                    start=False,
                                    stop=True,
                                )
                        # exp over both heads in one ACT op
                        ex = expp.tile([128, 2, 384], BF16)
                        nc.scalar.activation(
                            ex[:, :, 0:W],
                            sps[:, :, 0:W],
                            mybir.ActivationFunctionType.Exp,
                        )
                        # ctx + dn; q-tile pair (2m, 2m+1) shares one bank:
                        # [ctx_even | dn_even | ctx_odd | dn_odd] x 128
                        for j in range(nj):
                            qt = kt + j
                            m = qt // 2
                            if m not in cd_tiles:
                                cd_tiles[m] = pscd.tile(
                                    [128, 512], F32, tag="tcd", name=f"cd{m}"
                                )
                        for grp in ("ctx", "dn"):
                            for j in range(nj):
                                qt = kt + j
                                m = qt // 2
                                off = (qt % 2) * 256
                                dofs = 0 if grp == "ctx" else 128
                                cd = cd_tiles[m]
                                first = (
                                    grp == "ctx"
                                    and qt == 2 * m
                                    and kt == max(2 * m - 2, 0)
                                )
                                last = grp == "dn" and qt == 2 * m + 1 and kt == qt
                                for h in (0, 1):
                                    if grp == "ctx":
                                        lhsT = va[:, kt, h * 64 : h * 64 + 128]
                                    else:
                                        lhsT = on2[:, h * 64 : h * 64 + 128]
                                    nc.tensor.matmul(
                                        cd[:, off + dofs : off + dofs + 128],
                                        lhsT,
                                        ex[:, h, j * 128 : (j + 1) * 128],
                                        start=(first and h == 0),
                                        stop=(last and h == 1),
                                        skip_group_check=True,
                                    )
                        # finalize q-tile qt == kt
                        qt = kt
                        m = qt // 2
                        off = (qt % 2) * 256
                        cd = cd_tiles[m]
                        rec = recp.tile([128, 128], F32)
                        nc.vector.reciprocal(rec[:], cd[:, off + 128 : off + 256])
                        ctxn = ctxp.tile([128, 128], BF16)
                        nc.vector.tensor_mul(ctxn[:], cd[:, off : off + 128], rec[:])
                        if qt % 2 == 1 or qt == N_ST - 1:
                            del cd_tiles[m]
                        # out-projection + bias; both halves in one psum tile
                        osb = outp.tile([128, H], BF16)
                        ops = pssc.tile([128, 2, 512], F32, tag="tsc")
                        nc.tensor.matmul(
                            ops[:, 0, :], ctxn[:], wo_sb[:, 0:512], start=True, stop=True
                        )
                        nc.tensor.matmul(
                            ops[:, 1, :],
                            ctxn[:],
                            wo_sb[:, 512:1024],
                            start=True,
                            stop=True,
                        )
                        nc.vector.tensor_add(osb[:, 0:512], ops[:, 0, :], bo_sb[:, 0:512])
                        nc.scalar.copy(osb[:, 512:1024], ops[:, 1, :])
                        nc.gpsimd.tensor_add(
                            osb[:, 512:1024], osb[:, 512:1024], bo_sb[:, 512:1024]
                        )
                        nc.sync.dma_start(
                            out=out_ap[qt * 128 : (qt + 1) * 128, :], in_=osb[:]
                        )

                    # interleaved emission: chunk sc, then kts 4sc-2..4sc+1
                    for sc in range(N_SC):
                        emit_chunk(sc)
                        k_lo = max(4 * sc - 2, 0)
                        k_hi = 4 * sc + 1
                        for kt in range(k_lo, k_hi + 1):
                            emit_kt(kt)
                    for kt in (N_ST - 2, N_ST - 1):
                        emit_kt(kt)

    nc.compile()
    return nc


def build_in_maps(x, wq, wk, wv, wo, bo):
    xf = np.asarray(x, dtype=np.float32).reshape(S, H)
    # xT blocked: (p, kt, s) = x[s, kt*128 + p]
    xT_b = np.ascontiguousarray(xf.reshape(S, N_KT, 128).transpose(2, 1, 0).astype(BF))

    b = np.arange(128)[:, None]  # k within tile
    a = np.arange(128)[None, :]  # q within tile
    mask_a = np.where(b <= a, 0.0, NEG).astype(BF)  # diag tile (qt==kt)
    mask_b = np.where(b > a, 0.0, NEG).astype(BF)  # qt==kt+2 tile
    m2 = np.ascontiguousarray(np.concatenate([mask_a, mask_b], axis=1))
    ident = np.eye(128, dtype=np.float32).astype(BF)

    def blk(wT):  # [H, CD] -> [128, N_KT, CD]
        return np.ascontiguousarray(
            wT.reshape(N_KT, 128, CD).transpose(1, 0, 2).astype(BF)
        )

    in_maps = []
    for c in range(N_CORES):
        r0, r1 = c * CD, (c + 1) * CD
        bo_c = (bo if c == 0 else np.zeros_like(bo)).astype(np.float32)
        bo_b = np.ascontiguousarray(np.broadcast_to(bo_c, (128, H)).astype(BF))
        in_maps.append(
            {
                "xT_b": xT_b,
                "wq_b": blk(np.asarray(wq, np.float32)[r0:r1, :].T),
                "wk_b": blk(np.asarray(wk, np.float32)[r0:r1, :].T),
                "wv_b": blk(np.asarray(wv, np.float32)[r0:r1, :].T),
                "wo_b": np.ascontiguousarray(
                    np.asarray(wo, np.float32)[:, r0:r1].T.astype(BF)
                ),
                "bo_b": bo_b,
                "m2_b": m2,
                "id_b": ident,
            }
        )
    return in_maps


_NC_CACHE = None


def kernel(x, wq, wk, wv, wo, bo):
    global _NC_CACHE
    if _NC_CACHE is None:
        _NC_CACHE = build_program()
    nc = _NC_CACHE
    in_maps = build_in_maps(x, wq, wk, wv, wo, bo)
    res = run_bass_kernel_spmd(nc, in_maps, list(range(N_CORES)))
    out = res.results[0]["out"].astype(np.float64)
    for c in range(1, N_CORES):
        out += res.results[c]["out"].astype(np.float64)
    return out.reshape(1, S, H).astype(np.float32)
